# revision 13
# baseline (speedup 1.0000x reference)
"""Mamba2 (BareMambaLayer) forward on 8 Trainium2 cores via Bass/Tile.

Sharding: 8 shards = 4 batches x 2 sequence halves (2048 tokens each).
Each core runs the chunked-SSD scan (Q=128, 16 chunks) for all 32 heads on
its half-sequence; a paired AllGather hands the SSM state of the first half
to the second half (masked to zero on even cores).

Shapes (hardcoded per the problem spec):
  embed_data [4, 4096, 1024], W_in [4384, 1024], conv_w [2304, 4],
  conv_b [2304], dt_bias [32], A_log [32], D [32], norm_w [2048],
  W_out [1024, 2048]  ->  out [4, 4096, 1024] float32.
"""

import contextlib

import numpy as np
import ml_dtypes

import concourse.bass as bass
import concourse.tile as tile
from concourse import bacc, mybir
from concourse import bass_utils
from concourse.masks import make_identity, make_upper_triangular

F32 = mybir.dt.float32
BF16 = mybir.dt.bfloat16
AOP = mybir.AluOpType
AF = mybir.ActivationFunctionType
BF = ml_dtypes.bfloat16

NC = 8
DM = 1024          # d_model
DIN = 2048         # d_inner
NH = 32            # heads
HP = 64            # headdim
NS = 128           # d_state
Q = 128            # chunk len
LSH = 2048         # tokens per shard
NCH = LSH // Q     # 16 chunks
KD = DM // 128     # 8 k-tiles over d_model
MT = 19            # ceil(2336/128) M-tiles of W_A
KC = DIN // 128    # 16 k-tiles over d_inner
WTOT = KD * MT * 128 + KD * DIN + KC * DM  # 52224 packed weight columns

_CACHE = {}


def _emit(nc, D_vals):
    """Emit the whole per-core program. D_vals: python floats len 32."""
    ctx = contextlib.ExitStack()

    # ---------------- dram tensors ----------------
    emb_ap = nc.dram_tensor("emb", [128, KD * LSH], BF16, kind="ExternalInput").ap()
    preh_ap = nc.dram_tensor("preh", [128, MT * 3], BF16, kind="ExternalInput").ap()
    wpart_ap = nc.dram_tensor("wpart", [16, WTOT], BF16, kind="ExternalInput").ap()
    dtb_ap = nc.dram_tensor("dtb", [32, 1], F32, kind="ExternalInput").ap()
    acol_ap = nc.dram_tensor("acol", [32, 1], F32, kind="ExternalInput").ap()
    convw_ap = nc.dram_tensor("convw", [128, 72], F32, kind="ExternalInput").ap()
    convb_ap = nc.dram_tensor("convb", [128, 18], F32, kind="ExternalInput").ap()
    maskc_ap = nc.dram_tensor("maskc", [128, 1], F32, kind="ExternalInput").ap()
    out_ap = nc.dram_tensor("out", [LSH, DM], BF16, kind="ExternalOutput").ap()

    tc = ctx.enter_context(tile.TileContext(nc))

    dram = ctx.enter_context(tc.tile_pool(name="dram", bufs=1, space="DRAM"))
    yspill = dram.tile([LSH, DIN], BF16)
    zspill = dram.tile([LSH, DIN], BF16)
    elld = dram.tile([32, LSH], F32)
    hsnd = dram.tile([128, DIN], F32)
    hrcv = dram.tile([256, DIN], F32)
    wpin = dram.tile([16, WTOT], BF16)
    wall = dram.tile([128, WTOT], BF16)

    # ---------------- pools ----------------
    cpool = ctx.enter_context(tc.tile_pool(name="cpool", bufs=1))
    wpool = ctx.enter_context(tc.tile_pool(name="wpool", bufs=2))
    embp = ctx.enter_context(tc.tile_pool(name="embp", bufs=1))
    prep = ctx.enter_context(tc.tile_pool(name="prep", bufs=1))
    tailp = ctx.enter_context(tc.tile_pool(name="tailp", bufs=2))
    cxp = ctx.enter_context(tc.tile_pool(name="cxp", bufs=1))
    bcp = ctx.enter_context(tc.tile_pool(name="bcp", bufs=1))
    dtp = ctx.enter_context(tc.tile_pool(name="dtp", bufs=1))
    smp = ctx.enter_context(tc.tile_pool(name="smp", bufs=2))
    xtp = ctx.enter_context(tc.tile_pool(name="xtp", bufs=1))
    lrp = ctx.enter_context(tc.tile_pool(name="lrp", bufs=1))
    hdp = ctx.enter_context(tc.tile_pool(name="hdp", bufs=1))
    e2p = ctx.enter_context(tc.tile_pool(name="e2p", bufs=2))
    grp = ctx.enter_context(tc.tile_pool(name="grp", bufs=2))
    big16 = ctx.enter_context(tc.tile_pool(name="big16", bufs=3))
    big32 = ctx.enter_context(tc.tile_pool(name="big32", bufs=1))
    outp = ctx.enter_context(tc.tile_pool(name="outp", bufs=1))
    ps = ctx.enter_context(tc.tile_pool(name="ps", bufs=1, space="PSUM"))

    # ---------------- constants ----------------
    identf = cpool.tile([32, 32], F32)
    make_identity(nc, identf[:])
    identb = cpool.tile([128, 128], BF16)
    make_identity(nc, identb[:])
    triu = cpool.tile([128, 128], BF16)
    make_upper_triangular(nc, triu[:], val=1.0, diag=True)
    zcol = cpool.tile([128, 1], F32)
    nc.vector.memset(zcol[:], 0.0)
    epscol = cpool.tile([128, 1], F32)
    nc.vector.memset(epscol[:], 1e-5)
    ones32 = cpool.tile([32, 128], F32)
    nc.vector.memset(ones32[:], 1.0)

    dtb = cpool.tile([32, 1], F32)
    nc.sync.dma_start(dtb[:], dtb_ap[:])
    acol = cpool.tile([32, 1], F32)
    nc.sync.dma_start(acol[:], acol_ap[:])
    convw = cpool.tile([128, 72], F32)
    nc.sync.dma_start(convw[:], convw_ap[:])
    convb = cpool.tile([128, 18], F32)
    nc.sync.dma_start(convb[:], convb_ap[:])
    maskc = cpool.tile([128, 1], F32)
    nc.sync.dma_start(maskc[:], maskc_ap[:])
    preh = cpool.tile([128, MT * 3], BF16)
    nc.sync.dma_start(preh[:], preh_ap[:])

    # gather full weights from the 8-way sharded input
    nc.sync.dma_start(wpin[:], wpart_ap[:])
    nc.gpsimd.collective_compute(
        "AllGather", AOP.bypass,
        replica_groups=[[0, 1, 2, 3, 4, 5, 6, 7]],
        ins=[wpin.opt()], outs=[wall.opt()],
    )

    # ---------------- resident tiles ----------------
    emb = embp.tile([128, KD * LSH], BF16)
    for k in range(KD):
        nc.sync.dma_start(emb[:, k * LSH:(k + 1) * LSH],
                          emb_ap[:, k * LSH:(k + 1) * LSH])
    # weights share one 2-slot pool: wa0+wa1 (phase 1) then wz, wo
    WHALF = KD * MT * 128 // 2  # 9728
    wa0 = wpool.tile([128, WHALF], BF16, tag="w")
    nc.sync.dma_start(wa0[:], wall[:, :WHALF])
    wa1 = wpool.tile([128, WHALF], BF16, tag="w")
    nc.sync.dma_start(wa1[:], wall[:, WHALF:2 * WHALF])

    def wa_slice(k, m):
        idx = (k * MT + m) * 128
        if idx < WHALF:
            return wa0[:, idx:idx + 128]
        return wa1[:, idx - WHALF:idx - WHALF + 128]

    bres = bcp.tile([128, LSH], BF16)
    cres = bcp.tile([128, LSH], BF16)
    dtsp = dtp.tile([32, LSH], F32)      # raw dt, softplus'd in place
    ellc = dtp.tile([32, LSH], F32)
    lgp = dtp.tile([32, NCH], F32)
    hst = hdp.tile([128, DIN], F32)
    nc.vector.memset(hst[:], 0.0)

    # ---------------- P1..P4: per strip: in_proj -> dt/scan -> conv+scan ----
    pre_strips = []
    tails = []

    def emit_in_proj_strip(s):
        pre = prep.tile([128, MT * 512], BF16, name=f"pre{s}", tag="pre")
        tl = tailp.tile([128, 18 * 3], BF16, name=f"tail{s}", tag="tail")
        for m in range(MT):
            pmm = ps.tile([128, 512], F32, tag="mm", bufs=2)
            for k in range(KD):
                nc.tensor.matmul(pmm[:], wa_slice(k, m),
                                 emb[:, k * LSH + s * 512:k * LSH + (s + 1) * 512],
                                 start=(k == 0), stop=(k == KD - 1))
            nc.vector.tensor_copy(pre[:, m * 512:(m + 1) * 512], pmm[:])
            if m < 18:
                nc.vector.tensor_copy(tl[:, m * 3:(m + 1) * 3], pmm[:, 509:512])
            if m == MT - 1:
                nc.vector.tensor_copy(dtsp[:, s * 512:(s + 1) * 512], pmm[0:32, :])
        pre_strips.append(pre)
        tails.append(tl)

    def emit_dt_chunk(c):
        ch = slice(c * Q, (c + 1) * Q)
        sa = smp.tile([32, 128], F32, tag="sa")
        nc.vector.tensor_scalar(sa[:], dtsp[:, ch], acol[:], None, AOP.mult)
        nc.vector.tensor_tensor_scan(ellc[:, ch], ones32[:], sa[:], 0.0,
                                     AOP.mult, AOP.add)
        nc.sync.dma_start(elld[:, ch], ellc[:, ch])

    def emit_conv_chunk(c):
        ch = slice(c * Q, (c + 1) * Q)
        strip = c // 4
        cx = cxp.tile([128, 16 * 128], BF16, tag="cx")
        for m in range(18):
            acc = smp.tile([128, 128], F32, tag="cacc")
            for k in range(4):
                g0 = c * Q + k - 3
                pieces = []
                t = 0
                while t < 128:
                    g = g0 + t
                    if g < 0:
                        w = min(-g, 128 - t)
                        pieces.append((t, w,
                                       preh[:, m * 3 + g + 3:m * 3 + g + 3 + w]))
                    else:
                        sx = g // 512
                        off = g % 512
                        w = min(512 - off, 128 - t)
                        if sx < strip:
                            src = tails[sx][:, m * 3 + off - 509:
                                            m * 3 + off - 509 + w]
                        else:
                            src = pre_strips[sx][:, m * 512 + off:
                                                 m * 512 + off + w]
                        pieces.append((t, w, src))
                    t += w
                for (t, w, src) in pieces:
                    sc = convw[:, m * 4 + k:m * 4 + k + 1]
                    if k == 0:
                        nc.vector.tensor_scalar(acc[:, t:t + w], src, sc, None,
                                                AOP.mult)
                    else:
                        nc.vector.scalar_tensor_tensor(acc[:, t:t + w], src, sc,
                                                       acc[:, t:t + w],
                                                       AOP.mult, AOP.add)
            if m < 16:
                dest = cx[:, m * 128:(m + 1) * 128]
            elif m == 16:
                dest = bres[:, ch]
            else:
                dest = cres[:, ch]
            nc.scalar.activation(dest, acc[:], AF.Silu, bias=convb[:, m:m + 1])
        return cx

    def emit_scan_chunk(c, cx):
        ch = slice(c * Q, (c + 1) * Q)
        ptr = ps.tile([128, 128], F32, tag="tr", bufs=2)
        nc.tensor.transpose(ptr[:, 0:32], dtsp[:, ch], identf[:])
        nc.tensor.transpose(ptr[:, 32:64], ellc[:, ch], identf[:])
        wba = smp.tile([32, 128], F32, tag="wba")
        lamc = ellc[:, c * Q + 127:c * Q + 128]
        nc.vector.tensor_tensor(wba[:], lamc.to_broadcast((32, 128)), ellc[:, ch],
                                AOP.subtract)
        nc.tensor.transpose(ptr[:, 64:96], wba[:], identf[:])

        stl = smp.tile([128, 64], F32, tag="stl")       # dtT | ellT
        nc.vector.tensor_copy(stl[:], ptr[:, 0:64])
        wb = smp.tile([128, 32], F32, tag="wb")
        nc.scalar.activation(wb[:], ptr[:, 64:96], AF.Exp)
        ewt = smp.tile([128, 32], F32, tag="ewt")
        nc.scalar.activation(ewt[:], ptr[:, 32:64], AF.Exp)

        # eg: exp(lambda) replicated down partitions, from elld
        egr = smp.tile([128, 32], F32, tag="egr")
        src = bass.AP(tensor=elld.tensor, offset=elld.offset + c * Q + 127,
                      ap=[[0, 128], [LSH, 32]])
        nc.sync.dma_start(egr[:], src)
        eg = smp.tile([128, 32], F32, tag="eg")
        nc.scalar.activation(eg[:], egr[:], AF.Exp)

        # BT: transpose of B chunk
        pbt = ps.tile([128, 128], BF16, tag="tr", bufs=2)
        nc.tensor.transpose(pbt[:], bres[:, ch], identb[:])
        bt = smp.tile([128, 128], BF16, tag="bt")
        nc.vector.tensor_copy(bt[:], pbt[:])

        # G2 = B^T C, masked upper-triangular (s<=t)
        pg = ps.tile([128, 128], F32, tag="tr", bufs=2)
        nc.tensor.matmul(pg[:], bres[:, ch], cres[:, ch], start=True, stop=True)
        gm2 = smp.tile([128, 128], BF16, tag="gm2")
        nc.vector.tensor_tensor(gm2[:], pg[:], triu[:], AOP.mult)

        # xT: transpose conv-x chunk -> [t, ch]
        xt = xtp.tile([128, DIN], BF16, tag="xt")
        for j in range(16):
            pxt = ps.tile([128, 128], BF16, tag="tr", bufs=2)
            nc.tensor.transpose(pxt[:], cx[:, j * 128:(j + 1) * 128], identb[:])
            nc.vector.tensor_copy(xt[:, j * 128:(j + 1) * 128], pxt[:])

        ysb = big16.tile([128, DIN], BF16, tag="b16", name="ysb")
        for g in range(4):
            gs = slice(g * 512, (g + 1) * 512)
            lambf = grp.tile([128, 512], BF16, tag="lambf")
            nc.vector.tensor_copy(lambf[:], hst[:, gs])
            u2 = grp.tile([128, 512], BF16, tag="u2")
            for j in range(8):
                h = g * 8 + j
                nc.vector.tensor_scalar(u2[:, j * 64:(j + 1) * 64],
                                        xt[:, h * 64:(h + 1) * 64],
                                        stl[:, h:h + 1], None, AOP.mult)
            lrep = lrp.tile([128, 1024], F32, tag="lrep")
            for hv in range(2):
                src = bass.AP(tensor=elld.tensor,
                              offset=elld.offset + (g * 8 + hv * 4) * LSH + c * Q,
                              ap=[[0, 128], [LSH, 4], [1, 128]])
                nc.sync.dma_start(
                    lrep[:, hv * 512:(hv + 1) * 512]
                    .rearrange("p (j t) -> p j t", t=128), src)

            pyi = ps.tile([128, 512], F32, tag="yi", bufs=2)
            pyn = ps.tile([128, 512], F32, tag="yn1", bufs=1)
            pdh = ps.tile([128, 512], F32, tag="dh", bufs=1)
            for j in range(8):
                h = g * 8 + j
                js = slice(j * 64, (j + 1) * 64)
                e2a = e2p.tile([128, 128], F32, tag="e2a")
                nc.vector.scalar_tensor_tensor(
                    e2a[:], lrep[:, j * 128:(j + 1) * 128], stl[:, 32 + h:33 + h],
                    zcol[:].to_broadcast((128, 128)), AOP.subtract, AOP.min)
                e2 = e2p.tile([128, 128], F32, tag="e2")
                nc.scalar.activation(e2[:], e2a[:], AF.Exp)
                m2 = e2p.tile([128, 128], BF16, tag="m2")
                nc.vector.tensor_tensor(m2[:], gm2[:], e2[:], AOP.mult)
                bw2 = e2p.tile([128, 128], BF16, tag="bw2")
                nc.vector.tensor_scalar(bw2[:], bt[:], wb[:, h:h + 1], None,
                                        AOP.mult)
                nc.tensor.matmul(pyi[:, js], m2[:], u2[:, js],
                                 start=True, stop=True)
                nc.tensor.matmul(pyn[:, js], cres[:, ch], lambf[:, js],
                                 start=True, stop=True)
                nc.tensor.matmul(pdh[:, js], bw2[:], u2[:, js],
                                 start=True, stop=True)
            # assembly: ysb = (pyn * ewt) + D*xt + pyi
            ya = grp.tile([128, 512], F32, tag="ya")
            for j in range(8):
                h = g * 8 + j
                js = slice(j * 64, (j + 1) * 64)
                nc.vector.tensor_scalar(ya[:, js], pyn[:, js], ewt[:, h:h + 1],
                                        None, AOP.mult)
                nc.vector.scalar_tensor_tensor(ya[:, js],
                                               xt[:, h * 64:(h + 1) * 64],
                                               float(D_vals[h]), ya[:, js],
                                               AOP.mult, AOP.add)
                nc.vector.tensor_tensor(ysb[:, g * 512 + j * 64:
                                            g * 512 + (j + 1) * 64],
                                        ya[:, js], pyi[:, js], AOP.add)
            # H update: H = eg*H + dh
            nc.vector.tensor_tensor(
                hst[:, gs].rearrange("p (j o) -> p j o", o=64),
                hst[:, gs].rearrange("p (j o) -> p j o", o=64),
                eg[:, g * 8:(g + 1) * 8].rearrange("p (j o) -> p j o", o=1)
                .broadcast_to([128, 8, 64]),
                AOP.mult)
            nc.vector.tensor_tensor(hst[:, gs], hst[:, gs], pdh[:], AOP.add)
        nc.sync.dma_start(yspill[ch, :], ysb[:])

    for s in range(4):
        emit_in_proj_strip(s)
        dts = dtsp[:, s * 512:(s + 1) * 512]
        nc.scalar.activation(dts, dts, AF.Exp, bias=dtb[:])
        nc.scalar.activation(dts, dts, AF.Ln, bias=ones32[0:32, 0:1])
        for c in range(4 * s, 4 * s + 4):
            emit_dt_chunk(c)
        for c in range(4 * s, 4 * s + 4):
            cx = emit_conv_chunk(c)
            emit_scan_chunk(c, cx)

    # lgp: exclusive prefix over chunk log-decay totals
    lam_all = ellc[:, 127::128]  # [32, 16]
    lgi = smp.tile([32, NCH], F32, tag="sa")
    nc.vector.tensor_tensor_scan(lgi[:], ones32[:, 0:NCH], lam_all, 0.0,
                                 AOP.mult, AOP.add)
    nc.vector.tensor_sub(lgp[:], lgi[:], lam_all)

    # ---------------- Pz: z-projection, silu'd, spilled to DRAM ----------------
    wz = wpool.tile([128, KD * DIN], BF16, tag="w")
    nc.sync.dma_start(wz[:], wall[:, 2 * WHALF:2 * WHALF + KD * DIN])
    for c in range(NCH):
        zrow = big16.tile([128, DIN], BF16, tag="b16", name="zrow")
        for g in range(4):
            pz = ps.tile([128, 512], F32, tag="mm", bufs=2)
            for k in range(KD):
                nc.tensor.matmul(pz[:],
                                 emb[:, k * LSH + c * Q:k * LSH + (c + 1) * Q],
                                 wz[:, k * DIN + g * 512:k * DIN + (g + 1) * 512],
                                 start=(k == 0), stop=(k == KD - 1))
            nc.scalar.activation(zrow[:, g * 512:(g + 1) * 512], pz[:], AF.Silu)
        nc.sync.dma_start(zspill[c * Q:(c + 1) * Q, :], zrow[:])

    # ---------------- P5: state exchange ----------------
    nc.sync.dma_start(hsnd[:], hst[:])
    nc.gpsimd.collective_compute(
        "AllGather", AOP.bypass,
        replica_groups=[[0, 1], [2, 3], [4, 5], [6, 7]],
        ins=[hsnd.opt()], outs=[hrcv.opt()],
    )
    hin = big32.tile([128, DIN], F32, tag="b32", name="hin")
    nc.sync.dma_start(hin[:], hrcv[0:128, :])
    hinb = hdp.tile([128, DIN], BF16)
    nc.vector.tensor_scalar(hinb[:], hin[:], maskc[:], None, AOP.mult)

    wo = wpool.tile([128, KC * DM], BF16, tag="w")
    nc.sync.dma_start(wo[:], wall[:, 2 * WHALF + KD * DIN:])

    # ---------------- P6: correction + epilogue per chunk ----------------
    for c in range(NCH):
        ch = slice(c * Q, (c + 1) * Q)
        yread = big16.tile([128, DIN], BF16, tag="b16", name="yread")
        nc.sync.dma_start(yread[:], yspill[ch, :])
        zrd = big16.tile([128, DIN], BF16, tag="b16", name="zrd")
        nc.sync.dma_start(zrd[:], zspill[ch, :])
        yfull = big16.tile([128, DIN], BF16, tag="b16", name="yfull")
        ega = smp.tile([32, 128], F32, tag="wba")
        nc.vector.tensor_tensor(ega[:], lgp[:, c:c + 1].to_broadcast((32, 128)),
                                ellc[:, ch], AOP.add)
        ptr6 = ps.tile([128, 128], F32, tag="tr", bufs=2)
        nc.tensor.transpose(ptr6[:, 0:32], ega[:], identf[:])
        ewgtc = smp.tile([128, 32], F32, tag="ewt")
        nc.scalar.activation(ewgtc[:], ptr6[:, 0:32], AF.Exp)
        for g in range(4):
            gs = slice(g * 512, (g + 1) * 512)
            pyc = ps.tile([128, 512], F32, tag="yi", bufs=2)
            for j in range(8):
                h = g * 8 + j
                js = slice(j * 64, (j + 1) * 64)
                nc.tensor.matmul(pyc[:, js], cres[:, ch],
                                 hinb[:, h * 64:(h + 1) * 64],
                                 start=True, stop=True)
            for j in range(8):
                h = g * 8 + j
                nc.vector.scalar_tensor_tensor(
                    yfull[:, g * 512 + j * 64:g * 512 + (j + 1) * 64],
                    pyc[:, j * 64:(j + 1) * 64],
                    ewgtc[:, h:h + 1],
                    yread[:, g * 512 + j * 64:g * 512 + (j + 1) * 64],
                    AOP.mult, AOP.add)
            nc.vector.tensor_tensor(yfull[:, gs], yfull[:, gs], zrd[:, gs],
                                    AOP.mult)
        # rms norm over 2048
        ssq4 = smp.tile([128, 4], F32, tag="ssq4")
        for g in range(4):
            gs = slice(g * 512, (g + 1) * 512)
            sqd = grp.tile([128, 512], F32, tag="ya")
            nc.scalar.activation(sqd[:], yfull[:, gs], AF.Square,
                                 accum_out=ssq4[:, g:g + 1])
        ssq = smp.tile([128, 1], F32, tag="ssq")
        nc.vector.tensor_reduce(ssq[:], ssq4[:], mybir.AxisListType.X, AOP.add)
        sdev = smp.tile([128, 1], F32, tag="sdev")
        nc.scalar.activation(sdev[:], ssq[:], AF.Ln, bias=epscol[:],
                             scale=1.0 / DIN)
        rinv = smp.tile([128, 1], F32, tag="rinv")
        nc.scalar.activation(rinv[:], sdev[:], AF.Exp, scale=-0.5)
        yn = big16.tile([128, DIN], BF16, tag="b16", name="yn")
        nc.vector.tensor_scalar(yn[:], yfull[:], rinv[:], None, AOP.mult)
        ynt = big16.tile([128, DIN], BF16, tag="b16", name="ynt")
        for j in range(16):
            pyt = ps.tile([128, 128], BF16, tag="tr", bufs=2)
            nc.tensor.transpose(pyt[:], yn[:, j * 128:(j + 1) * 128], identb[:])
            nc.vector.tensor_copy(ynt[:, j * 128:(j + 1) * 128], pyt[:])
        osb = outp.tile([128, DM], BF16, tag="osb")
        for g in range(2):
            po = ps.tile([128, 512], F32, tag="mm", bufs=2)
            for kc in range(KC):
                nc.tensor.matmul(po[:], ynt[:, kc * 128:(kc + 1) * 128],
                                 wo[:, kc * DM + g * 512:kc * DM + (g + 1) * 512],
                                 start=(kc == 0), stop=(kc == KC - 1))
            nc.vector.tensor_copy(osb[:, g * 512:(g + 1) * 512], po[:])
        nc.sync.dma_start(out_ap[ch, :], osb[:])

    ctx.close()


def _build(D_vals):
    key = tuple(D_vals)
    if key in _CACHE:
        return _CACHE[key]
    nc = bacc.Bacc("TRN2", target_bir_lowering=False, debug=False, num_devices=NC)
    _emit(nc, D_vals)
    nc.compile()
    runner = _make_runner(nc)
    _CACHE[key] = runner
    return runner


def _make_runner(nc):
    """jit-once SPMD runner (replaces run_bass_kernel_spmd; axon/PJRT path)."""
    import jax
    import jax.numpy as jnp
    from jax.sharding import Mesh, PartitionSpec, NamedSharding
    from jax.experimental.shard_map import shard_map
    from concourse import bass2jax

    try:
        jax.config.update("jax_compilation_cache_dir", "/tmp/jax_kernel_cache")
        jax.config.update("jax_persistent_cache_min_entry_size_bytes", -1)
        jax.config.update("jax_persistent_cache_min_compile_time_secs", 0.0)
    except Exception:
        pass

    bass2jax.install_neuronx_cc_hook()
    partition_name = (nc.partition_id_tensor.name
                      if nc.partition_id_tensor else None)
    in_names, out_names, out_avals = [], [], []
    out_shapes = {}
    for alloc in nc.m.functions[0].allocations:
        if not isinstance(alloc, mybir.MemoryLocationSet):
            continue
        name = alloc.memorylocations[0].name
        if alloc.kind == "ExternalInput":
            if name != partition_name:
                in_names.append(name)
        elif alloc.kind == "ExternalOutput":
            shape = tuple(alloc.tensor_shape)
            dtype = mybir.dt.np(alloc.dtype)
            out_avals.append(jax.core.ShapedArray(shape, dtype))
            out_names.append(name)
            out_shapes[name] = (shape, dtype)
    n_params = len(in_names)
    in_names_full = in_names + out_names + (
        [partition_name] if partition_name else [])

    def _body(*args):
        operands = list(args)
        if partition_name is not None:
            operands.append(bass2jax.partition_id_tensor())
        outs = bass2jax._bass_exec_p.bind(
            *operands, out_avals=tuple(out_avals),
            in_names=tuple(in_names_full), out_names=tuple(out_names),
            lowering_input_output_aliases=(),
            sim_require_finite=True, sim_require_nnan=True, nc=nc)
        return tuple(outs)

    devices = jax.devices()[:NC]
    mesh = Mesh(np.asarray(devices), ("core",))
    n_outs = len(out_names)
    in_specs = (PartitionSpec("core"),) * (n_params + n_outs)
    out_specs = (PartitionSpec("core"),) * n_outs
    sharded = jax.jit(shard_map(_body, mesh=mesh, in_specs=in_specs,
                                out_specs=out_specs, check_rep=False),
                      keep_unused=True)
    # output placeholder buffers materialized on-device once (never read)
    shard = NamedSharding(mesh, PartitionSpec("core"))
    zeros_dev = [
        jax.jit(lambda shape=shape, dtype=dtype: jnp.zeros((NC * shape[0],) + shape[1:], dtype),
                out_shardings=shard)()
        for name, (shape, dtype) in ((n, out_shapes[n]) for n in out_names)
    ]
    for z in zeros_dev:
        z.block_until_ready()

    def run(in_maps):
        per_core = [[np.asarray(m[nm]) for nm in in_names] for m in in_maps]
        concat_in = [np.concatenate([per_core[c][i] for c in range(NC)], axis=0)
                     for i in range(n_params)]
        outs = sharded(*concat_in, *zeros_dev)
        results = []
        for c in range(NC):
            r = {}
            for i, nm in enumerate(out_names):
                shape, _ = out_shapes[nm]
                r[nm] = np.asarray(outs[i])[c * shape[0]:(c + 1) * shape[0]]
            results.append(r)
        return results

    return run


def kernel(embed_data, W_in, conv_w, conv_b, dt_bias, A_log, D, norm_w, W_out):
    embed_data = np.asarray(embed_data, np.float32)
    W_in = np.asarray(W_in, np.float32)
    conv_w = np.asarray(conv_w, np.float32)
    conv_b = np.asarray(conv_b, np.float32)
    dt_bias = np.asarray(dt_bias, np.float32)
    A_log = np.asarray(A_log, np.float32)
    D = np.asarray(D, np.float32)
    norm_w = np.asarray(norm_w, np.float32)
    W_out = np.asarray(W_out, np.float32)

    b, l, dm = embed_data.shape
    W_A = W_in[DIN:]                       # [2336, 1024]
    W_Z = W_in[:DIN]
    W_O = W_out * norm_w[None, :]
    A = -np.exp(A_log)

    # ---- pack weights (shared across cores) ----
    waT = np.zeros((DM, MT * 128), np.float32)
    waT[:, :W_A.shape[0]] = W_A.T
    wa_pack = np.ascontiguousarray(
        waT.reshape(KD, 128, MT, 128).transpose(1, 0, 2, 3).reshape(128, -1)
    ).astype(BF)
    wz_pack = np.ascontiguousarray(
        W_Z.T.reshape(KD, 128, DIN).transpose(1, 0, 2).reshape(128, -1)).astype(BF)
    wo_pack = np.ascontiguousarray(
        W_O.T.reshape(KC, 128, DM).transpose(1, 0, 2).reshape(128, -1)).astype(BF)
    wcat = np.concatenate([wa_pack, wz_pack, wo_pack], axis=1)  # [128, WTOT]
    convw_pack = np.ascontiguousarray(
        conv_w.reshape(18, 128, 4).transpose(1, 0, 2).reshape(128, 72))
    convb_pack = np.ascontiguousarray(conv_b.reshape(18, 128).T)

    in_maps = []
    for c in range(NC):
        bi, hf = c // 2, c % 2
        s0 = hf * LSH
        seg = embed_data[bi, s0:s0 + LSH]                  # [2048, 1024]
        embT = np.ascontiguousarray(
            seg.T.reshape(KD, 128, LSH).transpose(1, 0, 2).reshape(128, -1)
        ).astype(BF)
        # conv halo: in_proj of the 3 preceding tokens (zeros for first half)
        pre3 = np.zeros((3, MT * 128), np.float32)
        if hf == 1:
            pre3[:, :W_A.shape[0]] = embed_data[bi, s0 - 3:s0] @ W_A.T
        preh_pack = np.ascontiguousarray(
            pre3.T.reshape(MT, 128, 3).transpose(1, 0, 2).reshape(128, MT * 3)
        ).astype(BF)
        in_maps.append({
            "emb": embT,
            "preh": preh_pack,
            "wpart": np.ascontiguousarray(wcat[16 * c:16 * (c + 1)]),
            "dtb": dt_bias.reshape(32, 1),
            "acol": A.reshape(32, 1),
            "convw": convw_pack,
            "convb": convb_pack,
            "maskc": np.full((128, 1), float(hf), np.float32),
        })

    runner = _build([float(x) for x in D])
    results = runner(in_maps)

    out = np.empty((b, l, dm), np.float32)
    for c in range(NC):
        bi, hf = c // 2, c % 2
        out[bi, hf * LSH:(hf + 1) * LSH] = results[c]["out"].astype(np.float32)
    return out
